# revision 68
# baseline (speedup 1.0000x reference)
"""Trainium2 Bass kernel for nn_CNN2LWithRPE (transformer layer + CNN head).

Sharding: data-parallel over batch across 8 NeuronCores (2 batch rows each).
All parameters replicated. The only cross-core communication is two tiny
AllReduces for the training-mode BatchNorm statistics.

Per-core layout (B_loc batches, T = B_loc*L tokens):
  - activations transposed in SBUF as bf16 [D=128 part, T free] so every
    matmul runs at the PE's 1 cycle/row bf16 rate; PSUM accumulation, BN
    statistics and softmax denominators stay fp32.
  - attention as scores^T tiles [keys=128 part, queries=512 free]: per
    head h the PE quad row 32h is fed directly from the natural [D, T]
    layouts of kT/qT (tile_position=(32h, 0)).  The clipped-RPE bias is
    folded into a per-group exp bias constant plus narrow host-precomputed
    banded correction tiles added on DVE.
  - exp(scores) is written by the ACT engine directly as fp8e4 (values sit
    in [0.9, 1.1] so e4m3 quantization costs ~2e-3 relative on the
    attention output, well inside budget); PV then runs as a single fp8
    DoubleRow matmul per PAIR of key tiles (contraction 256 = 128
    partitions x 2 planes), halving PV stream time.  v carries an appended
    ones-column so the softmax denominator falls out of the same matmul;
    v is pre-scaled by 64 into fp8's normal range and the 1/64 is folded
    into the PSUM drain.
  - the PV matmul for group g is issued after exp(g+1) is enqueued
    (one-group lag) so the PE never stalls behind the ACT engine.
  - softmax reciprocal on DVE (reciprocal_approx_fast), not ACT, so the
    ACT engine keeps its exp table loaded through the attention phase.
  - layernorm in transposed layout: partition stats via ones-matmul,
    rstd = Exp(-0.5*Ln(var+eps)), rank-1 K=1 matmul broadcast back.
  - conv1d as K accumulating shifted matmuls.  Max-pools are hoisted
    before the BatchNorm affine (valid for bn gamma > 0), pooling /
    global-maxing directly out of PSUM, so BN apply collapses to a tiny
    per-channel affine after the AllReduce.
  - each batch's post-attention chain (out-proj, LN1, FFN, LN2, conv1) is
    emitted in half-sequence chunks interleaved under the remaining
    attention blocks, so the BN1 AllReduce fires almost immediately after
    the last attention block instead of after an exposed serial tail.
  - params ship as two packed DRAM blocks (one fp32 "small" block, one
    fp32 block that is bf16-cast on device in two big DVE ops), so
    compute starts a few microseconds into the kernel.
"""

import numpy as np

B, L = 16, 2048
NCAT, ED = 25, 120
D, H, HD = 128, 4, 32
FF = 256
MD = 32
C1, C2, K = 128, 256, 5
NC = 2
EPS = 1e-5
NCORES = 8
BLOC = B // NCORES
ISQ = float(1.0 / np.sqrt(HD))

QT = 512
KTILE = 128
VSC = 64.0  # v pre-scale into fp8 normal range

BAND_DELTAS = [-128, 0, 128, 256, 384, 512]
BAND_W = [32, 160, 288, 416, 512, 32]
BAND_C0 = [0, 0, 0, 0, 0, 480]
BAND_OFF = [0, 32, 192, 480, 896, 1408]
BAND_TOT = 1440

# packed fp32 small-param block column map
PF_W = 32
PC_IO25, PC_IO10, PC_QB, PC_KB, PC_OB = 0, 1, 2, 3, 4
PC_L1B, PC_L2B = 5, 7          # l1b: 5:7
PC_LN1G, PC_LN1B, PC_LN2G, PC_LN2B = 8, 9, 10, 11
PC_CEXP = 12                   # 12:20
PC_BN1G, PC_BN1B, PC_BN2G, PC_BN2B = 20, 21, 22, 24   # bn2g 22:24, bn2b 24:26
PC_FCB, PC_FCW = 26, 27        # fcw 27:31

# packed big-param block column map (bf16 element offsets; shipped as
# bit-packed fp32 halves and bitcast on device — no cast instructions)
PB_EMB, PB_PEMB, PB_INW, PB_IDENT = 0, 128, 256, 640
PB_WO, PB_L1W, PB_L2C, PB_C1W, PB_C2W = 768, 896, 1152, 1408, 2048
PB_BAND = 3328
PB_BC4 = PB_BAND + H * BAND_TOT   # 9088: head-band broadcast selector [H, D]
PB_SEL = PB_BC4 + D               # 9216: LN row-broadcast selectors [4, 4*D]
PB_W = PB_SEL + 4 * D             # 9728
PB_A = 768                     # first chunk (embed + qkv params + identity)


def _build(n_cores, bloc, lp, dbg=False):
    import contextlib
    import concourse.bass as bass
    import concourse.tile as tile
    from concourse import bacc, mybir

    f32 = mybir.dt.float32
    bf16 = mybir.dt.bfloat16
    f8 = mybir.dt.float8e4
    AF = mybir.ActivationFunctionType
    OP = mybir.AluOpType
    AX = mybir.AxisListType
    DR = mybir.MatmulPerfMode.DoubleRow

    T = bloc * lp
    NET = T // 512
    NQT = lp // QT
    NKT = lp // KTILE
    NG = NKT // 4
    NHG = NKT // 2            # half-groups = fp8 PV pairs
    NLT = lp // 512
    NPRB = lp // 256          # PV pairs per batch
    LP2 = lp // 2
    L2 = LP2 - (K - 1)
    LT2 = L2 // 2
    n1 = float(n_cores * bloc * lp)
    n2 = float(n_cores * bloc * L2)
    # query-tile halves (post-attn + tail chunk granularity)
    if NQT >= 2:
        HALVES = [list(range(NQT // 2)), list(range(NQT // 2, NQT))]
    else:
        HALVES = [[0]]

    nc = bacc.Bacc("TRN2", target_bir_lowering=False, debug=False,
                   num_devices=n_cores)

    def din(name, shape):
        return nc.dram_tensor(name, list(shape), f32, kind="ExternalInput")

    Xf = din("Xf", [T])
    saf = din("saf", [T])
    ptmf = din("ptmf", [T])
    pf_d = din("pf32", [D, PF_W])
    pba_d = din("pbig_a", [D, PB_A // 2])
    pbb_d = din("pbig_b", [D, (PB_W - PB_A) // 2])

    out_d = nc.dram_tensor("out", [bloc, NC], f32, kind="ExternalOutput")
    dbg_outs = {}
    if dbg:
        for nm, shp, dt_ in [("dbg_xT", [D, T], bf16), ("dbg_qT", [D, T], bf16),
                             ("dbg_kT", [D, T], bf16),
                             ("dbg_attnT", [D, T], bf16),
                             ("dbg_x1T", [D, T], bf16),
                             ("dbg_x2", [D, T], bf16),
                             ("dbg_feat", [C1, 2 * bloc], f32)]:
            dbg_outs[nm] = nc.dram_tensor(nm, shp, dt_, kind="ExternalOutput")

    def bc(ap1d, parts):
        return bass.AP(tensor=ap1d.tensor, offset=ap1d.offset,
                       ap=[[0, parts]] + [list(p) for p in ap1d.ap])

    with tile.TileContext(nc) as tc:
        ctx = contextlib.ExitStack()
        with ctx:
            pp = ctx.enter_context(tc.tile_pool(name="params", bufs=1))
            big = ctx.enter_context(tc.tile_pool(name="big", bufs=1))
            wk = ctx.enter_context(tc.tile_pool(name="wk", bufs=12))
            longs = ctx.enter_context(tc.tile_pool(name="longs", bufs=1))
            mid = ctx.enter_context(tc.tile_pool(name="mid", bufs=6))
            rowp = ctx.enter_context(tc.tile_pool(name="rows", bufs=4))
            ptp = ctx.enter_context(tc.tile_pool(name="pt", bufs=5))
            ps_sc = ctx.enter_context(tc.tile_pool(name="ps_sc", bufs=2, space="PSUM"))
            ps_sm = ctx.enter_context(tc.tile_pool(name="ps_sm", bufs=4, space="PSUM"))
            dram = ctx.enter_context(tc.tile_pool(name="dram", bufs=1, space="DRAM"))

            def wkt(p=D, f=512, dt=f32):
                return wk.tile([p, f], dt, tag="wk", name="wkt")

            def midt(p, f):
                return mid.tile([p, f], f32, tag="mid", name="midt")

            # ---- packed params: fp32 small block + bit-packed bf16 block ----
            pf = pp.tile([D, PF_W], f32, tag="pf", name="pf")
            nc.sync.dma_start(out=pf, in_=pf_d[:, :])
            pbt = pp.tile([D, PB_W // 2], f32, tag="pbt", name="pbt")
            nc.sync.dma_start(out=pbt[:, 0:PB_A // 2], in_=pba_d[:, :])
            pb16 = pbt[:, :].bitcast(bf16)

            io25 = pf[0:NCAT, PC_IO25:PC_IO25 + 1]
            io10 = pf[0:10, PC_IO10:PC_IO10 + 1]
            qb_s = pf[:, PC_QB:PC_QB + 1]
            kb_s = pf[:, PC_KB:PC_KB + 1]
            ob_s = pf[:, PC_OB:PC_OB + 1]
            l1b_s = pf[:, PC_L1B:PC_L1B + 2]
            l2b_s = pf[:, PC_L2B:PC_L2B + 1]
            ln1g = pf[:, PC_LN1G:PC_LN1G + 1]
            ln1b = pf[:, PC_LN1B:PC_LN1B + 1]
            ln2g = pf[:, PC_LN2G:PC_LN2G + 1]
            ln2b = pf[:, PC_LN2B:PC_LN2B + 1]
            cexp = pf[:, PC_CEXP:PC_CEXP + 2 * H]
            bn1g = pf[:, PC_BN1G:PC_BN1G + 1]
            bn1b = pf[:, PC_BN1B:PC_BN1B + 1]
            bn2g = pf[:, PC_BN2G:PC_BN2G + 2]
            bn2b = pf[:, PC_BN2B:PC_BN2B + 2]
            fcb_s = pf[0:NC, PC_FCB:PC_FCB + 1]
            fcwT = pf[:, PC_FCW:PC_FCW + 2 * NC]

            emb_s = pb16[0:NCAT, PB_EMB:PB_EMB + D]
            pemb_s = pb16[0:10, PB_PEMB:PB_PEMB + D]
            inwT = pb16[:, PB_INW:PB_INW + 3 * D]
            ident = pb16[:, PB_IDENT:PB_IDENT + D]
            woT = pb16[:, PB_WO:PB_WO + D]
            l1wT = pb16[:, PB_L1W:PB_L1W + FF]
            l2cat = pb16[:, PB_L2C:PB_L2C + 2 * D]
            c1wT = pb16[:, PB_C1W:PB_C1W + K * C1]
            c2wT = pb16[:, PB_C2W:PB_C2W + K * C2]
            band16 = pb16[:, PB_BAND:PB_BAND + H * BAND_TOT]
            bc4 = pb16[0:H, PB_BC4:PB_BC4 + D]
            sel4 = pb16[0:4, PB_SEL:PB_SEL + 4 * D]

            ones128 = pp.tile([D, 1], bf16, tag="ones128")
            nc.vector.memset(ones128, 1.0)
            eps128 = pp.tile([D, 1], f32, tag="eps128")
            nc.vector.memset(eps128, EPS)

            # ---- persistent activations ----
            xT = big.tile([D, T], bf16, tag="chA")
            qT = big.tile([D, T], bf16, tag="chE")
            kT = big.tile([D, T], bf16, tag="chB")
            # fp8 v, pair-packed for DoubleRow: [tok128, pair, plane, h*36+d]
            v8 = big.tile([D, T // 256, 2, H * 36], f8, tag="chC")
            attnT = big.tile([D, T], f32, tag="chD")
            attnB = big.tile([D, T], bf16, tag="chH")
            x1T = big.tile([D, T], bf16, tag="chG")
            r1T = big.tile([D, T], bf16, tag="chI")
            x2pad = big.tile([D, bloc * (lp + 4)], bf16, tag="chJ")
            r2T = big.tile([D, T], bf16, tag="chK")
            p1_sb = big.tile([C1, bloc * LP2], bf16, tag="chM")
            v8r = v8.rearrange("p pr two (h c) -> p pr two h c", c=36)
            nc.vector.memset(v8r[:, :, :, :, HD:HD + 1], VSC)
            nc.vector.memset(x2pad[:, :], 0.0)

            # ================= embedding + qkv (per tile) =================
            ebp = ctx.enter_context(tc.tile_pool(name="ebp", bufs=12))

            def embed_load(e):
                sl = slice(e * 512, (e + 1) * 512)
                xb = ebp.tile([NCAT, 512], f32, tag="eb", name="xb")
                nc.sync.dma_start(out=xb, in_=bc(Xf[sl], NCAT))
                sb_ = ebp.tile([NCAT, 512], f32, tag="eb", name="sb_")
                nc.sync.dma_start(out=sb_, in_=bc(saf[sl], NCAT))
                pb = ebp.tile([10, 512], f32, tag="eb", name="pb")
                nc.sync.dma_start(out=pb, in_=bc(ptmf[sl], 10))
                return xb, sb_, pb

            def embed_tile(e, loaded=None):
                sl = slice(e * 512, (e + 1) * 512)
                xb, sb_, pb = loaded if loaded is not None else embed_load(e)
                oh = wkt(NCAT, dt=bf16)
                nc.vector.tensor_scalar(out=oh, in0=xb, scalar1=io25,
                                        scalar2=None, op0=OP.is_equal)
                nc.vector.tensor_mul(oh, oh, sb_)
                ohp = wkt(10, dt=bf16)
                nc.vector.tensor_scalar(out=ohp, in0=pb, scalar1=io10,
                                        scalar2=None, op0=OP.is_equal)
                pe = ps_sm.tile([D, 512], f32, tag="sm")
                nc.tensor.matmul(pe, pemb_s, ohp, start=True, stop=False)
                nc.tensor.matmul(pe, emb_s, oh, start=False, stop=True)
                nc.vector.tensor_copy(xT[:, sl], pe)

            def qkv_tile(e):
                sl = slice(e * 512, (e + 1) * 512)
                pq = ps_sm.tile([D, 512], f32, tag="sm")
                nc.tensor.matmul(pq, inwT[:, 0:D], xT[:, sl],
                                 start=True, stop=True)
                nc.vector.tensor_scalar(out=qT[:, sl], in0=pq, scalar1=ISQ,
                                        scalar2=qb_s, op0=OP.mult, op1=OP.add)
                yield
                pk = ps_sm.tile([D, 512], f32, tag="sm")
                nc.tensor.matmul(pk, inwT[:, D:2 * D], xT[:, sl],
                                 start=True, stop=True)
                nc.vector.tensor_scalar(out=kT[:, sl], in0=pk, scalar1=kb_s,
                                        scalar2=None, op0=OP.add)
                yield
                for sub in range(4):
                    tt = (e * 512) // KTILE + sub
                    pv = ps_sm.tile([KTILE, D], f32, tag="sm")
                    nc.tensor.matmul(pv, xT[:, e * 512 + sub * KTILE:
                                            e * 512 + (sub + 1) * KTILE],
                                     inwT[:, 2 * D:3 * D],
                                     start=True, stop=True)
                    nc.vector.tensor_scalar(
                        out=v8r[:, tt // 2, tt % 2, :, 0:HD],
                        in0=pv.rearrange("p (h d) -> p h d", h=H),
                        scalar1=VSC, scalar2=None, op0=OP.mult)
                    yield

            def embed_qkv_gen(tiles, preloaded):
                for e, ld in zip(tiles, preloaded):
                    embed_tile(e, ld)
                    yield
                for e in tiles:
                    yield from qkv_tile(e)

            NETB = NET // bloc
            # batch 0 inputs first, then the big param chunk, so embedding
            # starts as early as possible
            preload0 = [embed_load(e) for e in range(NETB)]
            nc.sync.dma_start(out=pbt[:, PB_A // 2:], in_=pbb_d[:, :])

            # dummy tiny AllReduce: pays the first-collective trigger setup
            # (~11.5us) far from the critical path
            dmy = wk.tile([1, 1], f32, tag="wk", name="dmy")
            nc.vector.memset(dmy, 0.0)
            dmy_i = dram.tile([1, 1], f32, tag="dmyi")
            dmy_o = dram.tile([1, 1], f32, tag="dmyo")
            nc.sync.dma_start(out=dmy_i, in_=dmy)
            nc.gpsimd.collective_compute(
                "AllReduce", OP.add, replica_groups=[list(range(n_cores))],
                ins=[dmy_i[:, :].opt()], outs=[dmy_o[:, :].opt()])

            for e in range(NETB):
                embed_tile(e, preload0[e])
            # pre-issue batch 1's (dynamic) input DMAs so the pumped embed
            # compute never parks a DMA wait at the DVE queue head
            preload1 = [embed_load(e) for e in range(NETB, NET)] \
                if bloc > 1 else []
            for e in range(NETB):
                for _ in qkv_tile(e):
                    pass

            # ================= attention =================
            den32s = [[longs.tile([H, 512], f32, tag="den32_%d_%d" % (b, q),
                                  name="den32_%d_%d" % (b, q))
                       for q in range(NQT)] for b in range(bloc)]
            recips = [[longs.tile([H, 512], bf16, tag="recip_%d_%d" % (b, q),
                                  name="recip_%d_%d" % (b, q))
                       for q in range(NQT)] for b in range(bloc)]
            dmap = {(-1, 3): 0, (0, 0): 1, (0, 1): 2, (0, 2): 3, (0, 3): 4, (1, 0): 5}

            def attn_pair(b_, hp, qt):
                # two heads share one iteration; both PV accumulators live in
                # a single PSUM bank (h1 via column tile_position 64)
                hs = (2 * hp, 2 * hp + 1)
                ppv = {h: ps_sm.tile([HD + 1, 512], f32, tag="sm",
                                     name="ppv%d" % h) for h in hs}
                prev = {}

                def pv_mm(h, pt8, g2):
                    nc.tensor.matmul(
                        ppv[h], v8r[:, b_ * NPRB + g2, :, h, 0:HD + 1],
                        pt8.rearrange("p (two n) -> p two n", two=2),
                        start=(g2 == 0), stop=(g2 == NHG - 1),
                        perf_mode=DR)

                for g2 in range(NHG):
                    g = g2 // 2
                    side = 0 if g <= qt else 1
                    cur = {}
                    for h in hs:
                        sc = ps_sc.tile([D, 1024], f32, tag="sc",
                                        name="sc%d" % h)
                        for q2 in range(2):
                            kt = 2 * g2 + q2
                            nc.tensor.matmul(
                                sc[:, q2 * 512:(q2 + 1) * 512],
                                kT[32 * h:32 * h + 32,
                                   b_ * lp + kt * KTILE:
                                   b_ * lp + (kt + 1) * KTILE],
                                qT[32 * h:32 * h + 32,
                                   b_ * lp + qt * QT:b_ * lp + (qt + 1) * QT],
                                start=True, stop=True,
                                tile_position=(32 * h, 0))
                        for q2 in range(2):
                            q = (2 * g2 + q2) % 4
                            di = dmap.get((g - qt, q))
                            if di is not None:
                                # banded RPE correction on DVE; the one-group
                                # PV lag keeps it off the PE critical path
                                c0, w = BAND_C0[di], BAND_W[di]
                                nc.vector.tensor_tensor(
                                    out=sc[:, q2 * 512 + c0:q2 * 512 + c0 + w],
                                    in0=sc[:, q2 * 512 + c0:q2 * 512 + c0 + w],
                                    in1=band16[:, h * BAND_TOT + BAND_OFF[di]:
                                               h * BAND_TOT + BAND_OFF[di] + w],
                                    op=OP.add)
                        pt8 = ptp.tile([D, 1024], f8, tag="pt")
                        nc.scalar.activation(pt8, sc, AF.Exp,
                                             bias=cexp[:, 2 * h + side:
                                                       2 * h + side + 1],
                                             scale=1.0)
                        cur[h] = (pt8, g2)
                    for h in hs:
                        if h in prev:
                            pv_mm(h, *prev[h])
                        prev[h] = cur[h]
                for h in hs:
                    pv_mm(h, *prev[h])
                for h in hs:
                    pv_sb = wk.tile([HD + 1, 512], f32, tag="wk",
                                    name="pv_sb")
                    nc.vector.tensor_scalar(out=pv_sb, in0=ppv[h],
                                            scalar1=1.0 / VSC,
                                            scalar2=None, op0=OP.mult)
                    nc.sync.dma_start(
                        out=attnT[32 * h:32 * h + 32,
                                  b_ * lp + qt * QT:b_ * lp + (qt + 1) * QT],
                        in_=pv_sb[0:HD, :])
                    nc.sync.dma_start(
                        out=den32s[b_][qt][h:h + 1, :],
                        in_=pv_sb[HD:HD + 1, :])

            def post_attn_qt(b_, qt):
                rcf = wk.tile([H, 512], f32, tag="wk", name="rcf")
                nc.vector.reciprocal_approx_fast(rcf, den32s[b_][qt])
                nc.vector.tensor_copy(recips[b_][qt], rcf)
                # all four heads' 1/den rows broadcast in ONE selector matmul
                bcp = ps_sm.tile([D, 512], f32, tag="sm")
                nc.tensor.matmul(bcp, bc4, recips[b_][qt],
                                 start=True, stop=True)
                sl = slice(b_ * lp + qt * QT, b_ * lp + (qt + 1) * QT)
                nc.vector.tensor_mul(attnB[:, sl], attnT[:, sl], bcp)

            # ======== layernorm helper (transposed layout, generator) ========
            def layernorm_T(src, dst_fn, g_s, b_s, b_base, tiles):
                nt = len(tiles)
                s1c = midt(nt, 512)
                s2c = midt(nt, 512)
                for i, t_ in enumerate(tiles):
                    sl = slice(b_base * lp + t_ * 512, b_base * lp + (t_ + 1) * 512)
                    sq = wkt(dt=bf16)
                    nc.vector.tensor_mul(sq, src[:, sl], src[:, sl])
                    p1_ = ps_sm.tile([1, 512], f32, tag="sm")
                    nc.tensor.matmul(p1_, ones128, src[:, sl],
                                     start=True, stop=True)
                    s1t = rowp.tile([1, 512], f32, tag="row", name="s1t")
                    nc.vector.tensor_copy(s1t, p1_)
                    nc.sync.dma_start(out=s1c[i:i + 1, :], in_=s1t)
                    yield
                    p2_ = ps_sm.tile([1, 512], f32, tag="sm")
                    nc.tensor.matmul(p2_, ones128, sq, start=True, stop=True)
                    s2t = rowp.tile([1, 512], f32, tag="row", name="s2t")
                    nc.vector.tensor_copy(s2t, p2_)
                    nc.sync.dma_start(out=s2c[i:i + 1, :], in_=s2t)
                    yield
                m_ = wkt(nt)
                nc.vector.tensor_scalar(out=m_, in0=s1c, scalar1=1.0 / D,
                                        scalar2=None, op0=OP.mult)
                var = wkt(nt)
                nc.vector.tensor_scalar(out=var, in0=s2c, scalar1=1.0 / D,
                                        scalar2=None, op0=OP.mult)
                msq = wkt(nt)
                nc.vector.tensor_mul(msq, m_, m_)
                nc.vector.tensor_tensor(out=var, in0=var, in1=msq, op=OP.subtract)
                lnv = wkt(nt)
                nc.scalar.activation(lnv, var, AF.Ln, bias=eps128[0:nt, :],
                                     scale=1.0)
                rstd = wkt(nt, dt=bf16)
                nc.scalar.activation(rstd, lnv, AF.Exp, bias=0.0, scale=-0.5)
                mr = wkt(nt, dt=bf16)
                nc.vector.tensor_mul(mr, m_, rstd)
                yield
                for i, t_ in enumerate(tiles):
                    sl = slice(b_base * lp + t_ * 512, b_base * lp + (t_ + 1) * 512)
                    br = ps_sm.tile([D, 512], f32, tag="sm")
                    nc.tensor.matmul(br, sel4[0:nt, i * D:(i + 1) * D],
                                     rstd, start=True, stop=True)
                    tmp = wkt()
                    nc.vector.tensor_mul(tmp, src[:, sl], br)
                    yield
                    bm = ps_sm.tile([D, 512], f32, tag="sm")
                    nc.tensor.matmul(bm, sel4[0:nt, i * D:(i + 1) * D],
                                     mr, start=True, stop=True)
                    nc.vector.tensor_tensor(out=tmp, in0=tmp, in1=bm,
                                            op=OP.subtract)
                    nc.vector.tensor_scalar(out=dst_fn(t_), in0=tmp, scalar1=g_s,
                                            scalar2=b_s, op0=OP.mult, op1=OP.add)
                    yield

            bnst1 = longs.tile([C1, bloc * NLT, 6], f32, tag="bnst1")

            def conv1_tile(b_, t_):
                pc = ps_sm.tile([C1, 512], f32, tag="sm")
                for k_ in range(K):
                    nc.tensor.matmul(
                        pc, c1wT[:, k_ * C1:(k_ + 1) * C1],
                        x2pad[:, b_ * (lp + 4) + t_ * 512 + k_:
                              b_ * (lp + 4) + t_ * 512 + k_ + 512],
                        start=(k_ == 0), stop=(k_ == K - 1))
                nc.vector.bn_stats(out=bnst1[:, b_ * NLT + t_, :], in_=pc)
                cs_ = wkt(dt=bf16)
                nc.vector.tensor_copy(cs_, pc)
                cs2 = cs_.rearrange("p (l two) -> p l two", two=2)
                nc.vector.tensor_tensor(
                    out=p1_sb[:, b_ * LP2 + t_ * 256:b_ * LP2 + (t_ + 1) * 256],
                    in0=cs2[:, :, 0], in1=cs2[:, :, 1], op=OP.max)

            # ---- tail chunk: out-proj+LN1+FFN+LN2 (+conv1 tiles) ----
            def tail_part(b_, tiles, conv_tiles):
                for qt in tiles:
                    sl = slice(b_ * lp + qt * QT, b_ * lp + (qt + 1) * QT)
                    po = ps_sm.tile([D, 512], f32, tag="sm")
                    nc.tensor.matmul(po, woT, attnB[:, sl],
                                     start=True, stop=True)
                    nc.vector.tensor_scalar(out=r1T[:, sl], in0=po, scalar1=ob_s,
                                            scalar2=None, op0=OP.add)
                    nc.vector.tensor_tensor(out=r1T[:, sl], in0=r1T[:, sl],
                                            in1=xT[:, sl], op=OP.add)
                    yield
                yield from layernorm_T(
                    r1T,
                    lambda t_, b0=b_: x1T[:, b0 * lp + t_ * 512:
                                          b0 * lp + (t_ + 1) * 512],
                    ln1g, ln1b, b_, tiles)
                for qt in tiles:
                    sl = slice(b_ * lp + qt * QT, b_ * lp + (qt + 1) * QT)
                    h1a = wkt(dt=bf16)
                    h1b = wkt(dt=bf16)
                    for half, dest in ((0, h1a), (1, h1b)):
                        ph = ps_sm.tile([D, 512], f32, tag="sm")
                        nc.tensor.matmul(ph, l1wT[:, half * D:(half + 1) * D],
                                         x1T[:, sl], start=True, stop=True)
                        nc.vector.tensor_scalar(out=dest, in0=ph,
                                                scalar1=l1b_s[:, half:half + 1],
                                                scalar2=0.0, op0=OP.add,
                                                op1=OP.max)
                        yield
                    py = ps_sm.tile([D, 512], f32, tag="sm")
                    nc.tensor.matmul(py, l2cat[:, 0:D], h1a,
                                     start=True, stop=False)
                    nc.tensor.matmul(py, l2cat[:, D:2 * D], h1b,
                                     start=False, stop=True)
                    nc.vector.tensor_scalar(out=r2T[:, sl], in0=py, scalar1=l2b_s,
                                            scalar2=None, op0=OP.add)
                    nc.vector.tensor_tensor(out=r2T[:, sl], in0=r2T[:, sl],
                                            in1=x1T[:, sl], op=OP.add)
                    yield
                yield from layernorm_T(
                    r2T,
                    lambda t_, b0=b_: x2pad[:, b0 * (lp + 4) + 2 + t_ * 512:
                                            b0 * (lp + 4) + 2 + (t_ + 1) * 512],
                    ln2g, ln2b, b_, tiles)
                # conv1 + bn1 stats + maxpool (pre-BN pool: bn gamma > 0)
                for t_ in conv_tiles:
                    conv1_tile(b_, t_)
                    yield

            # ---- attention + interleaved tails (both batches) ----
            from collections import deque
            pending = deque()

            def pump(n):
                for _ in range(n):
                    while pending:
                        try:
                            next(pending[0])
                            break
                        except StopIteration:
                            pending.popleft()

            if bloc > 1:
                eq1 = embed_qkv_gen(range(NETB, NET), preload1)
                pending.append(eq1)
            for b_ in range(bloc):
                if b_ == 1:
                    # batch 1 attention needs its embed/qkv complete
                    for _ in eq1:
                        pass
                for qt in range(NQT):
                    for hi_p, hp in enumerate(range(H // 2)):
                        attn_pair(b_, hp, qt)
                        if hi_p == 0 and qt > 0:
                            # normalize the previous query tile one block
                            # late, off the qt boundary
                            post_attn_qt(b_, qt - 1)
                        pump(16)
                    if qt == NQT - 1:
                        post_attn_qt(b_, qt)
                    if b_ == 0 or bloc == 1:
                        # batch 0's tail hides under batch 1's attention:
                        # one full-width chunk (fewest ACT table switches)
                        if qt == NQT - 1:
                            pending.append(tail_part(
                                b_, list(range(NQT)), list(range(NLT))))
                    else:
                        # batch 1: shrinking chunks so the final exposed
                        # chain (feeding the BN1 AllReduce) is minimal
                        if NQT == 1:
                            pending.append(tail_part(b_, [0], [0]))
                        elif qt == 1:
                            pending.append(tail_part(b_, [0, 1], [0]))
                        elif qt == 2:
                            pending.append(tail_part(b_, [2], [1]))
                        elif qt == NQT - 1:
                            pending.append(tail_part(
                                b_, [qt], list(range(qt - 1, NLT))))
            while pending:
                for _ in pending.popleft():
                    pass

            if dbg:
                nc.sync.dma_start(out=dbg_outs["dbg_xT"][:, :], in_=xT)
                nc.sync.dma_start(out=dbg_outs["dbg_qT"][:, :], in_=qT)
                nc.sync.dma_start(out=dbg_outs["dbg_kT"][:, :], in_=kT)
                nc.sync.dma_start(out=dbg_outs["dbg_attnT"][:, :], in_=attnB)
                nc.sync.dma_start(out=dbg_outs["dbg_x1T"][:, :], in_=x1T)
                for b_ in range(bloc):
                    nc.sync.dma_start(
                        out=dbg_outs["dbg_x2"][:, b_ * lp:(b_ + 1) * lp],
                        in_=x2pad[:, b_ * (lp + 4) + 2:b_ * (lp + 4) + 2 + lp])

            # ================= bn1 allreduce + apply =================
            mv1 = wk.tile([C1, 2], f32, tag="wk")
            nc.vector.bn_aggr(out=mv1, in_=bnst1)
            part1 = wk.tile([C1, 2], f32, tag="wk")
            sqm = wk.tile([C1, 1], f32, tag="wk")
            nc.vector.tensor_mul(sqm, mv1[:, 0:1], mv1[:, 0:1])
            nc.vector.tensor_tensor(out=sqm, in0=sqm, in1=mv1[:, 1:2], op=OP.add)
            nl_ = float(bloc * lp)
            nc.vector.tensor_scalar(out=part1[:, 0:1], in0=mv1[:, 0:1],
                                    scalar1=nl_, scalar2=None, op0=OP.mult)
            nc.vector.tensor_scalar(out=part1[:, 1:2], in0=sqm,
                                    scalar1=nl_, scalar2=None, op0=OP.mult)
            bn1_in = dram.tile([C1, 2], f32, tag="bn1i")
            bn1_out = dram.tile([C1, 2], f32, tag="bn1o")
            nc.sync.dma_start(out=bn1_in, in_=part1)
            nc.gpsimd.collective_compute(
                "AllReduce", OP.add, replica_groups=[list(range(n_cores))],
                ins=[bn1_in[:, :].opt()], outs=[bn1_out[:, :].opt()])
            glob1 = wk.tile([C1, 2], f32, tag="wk")
            nc.sync.dma_start(out=glob1, in_=bn1_out)

            def bn_scale_shift(globc, n_, g_ap, b_ap):
                mean = wk.tile([C1, 1], f32, tag="wk")
                nc.vector.tensor_scalar(out=mean, in0=globc[:, 0:1],
                                        scalar1=1.0 / n_, scalar2=None, op0=OP.mult)
                ex2 = wk.tile([C1, 1], f32, tag="wk")
                nc.vector.tensor_scalar(out=ex2, in0=globc[:, 1:2],
                                        scalar1=1.0 / n_, scalar2=None, op0=OP.mult)
                msq_ = wk.tile([C1, 1], f32, tag="wk")
                nc.vector.tensor_mul(msq_, mean, mean)
                nc.vector.tensor_tensor(out=ex2, in0=ex2, in1=msq_, op=OP.subtract)
                lnv_ = wk.tile([C1, 1], f32, tag="wk")
                nc.scalar.activation(lnv_, ex2, AF.Ln, bias=eps128, scale=1.0)
                rstd_ = wk.tile([C1, 1], f32, tag="wk")
                nc.scalar.activation(rstd_, lnv_, AF.Exp, bias=0.0, scale=-0.5)
                scale = longs.tile([C1, 1], f32, tag="bnsc")
                nc.vector.tensor_mul(scale, rstd_, g_ap)
                shift = longs.tile([C1, 1], f32, tag="bnsh")
                nc.vector.tensor_mul(shift, mean, scale)
                nc.vector.tensor_tensor(out=shift, in0=b_ap, in1=shift,
                                        op=OP.subtract)
                return scale, shift

            sc1, sh1 = bn_scale_shift(glob1, n1, bn1g, bn1b)
            # BN1 affine + relu applied in place on the (pre-BN) pooled map
            nc.vector.tensor_scalar(out=p1_sb, in0=p1_sb,
                                    scalar1=sc1, scalar2=sh1,
                                    op0=OP.mult, op1=OP.add)
            nc.vector.tensor_scalar(out=p1_sb, in0=p1_sb,
                                    scalar1=0.0, scalar2=None, op0=OP.max)

            # ================= conv2 + bn2 (global max from PSUM) ========
            bnst2 = longs.tile([C1, 2, bloc * 2, 6], f32, tag="bnst2")
            gmax = longs.tile([C1, 2, bloc, 2], f32, tag="gmax")
            for b_ in range(bloc):
                for half in range(2):
                    for t_ in range(2):
                        pc = ps_sm.tile([C1, LT2], f32, tag="sm")
                        for k_ in range(K):
                            nc.tensor.matmul(
                                pc, c2wT[:, k_ * C2 + half * C1:
                                         k_ * C2 + (half + 1) * C1],
                                p1_sb[:, b_ * LP2 + t_ * LT2 + k_:
                                      b_ * LP2 + t_ * LT2 + k_ + LT2],
                                start=(k_ == 0), stop=(k_ == K - 1))
                        nc.vector.bn_stats(out=bnst2[:, half, b_ * 2 + t_, :],
                                           in_=pc)
                        nc.vector.reduce_max(
                            out=gmax[:, half, b_, t_:t_ + 1], in_=pc, axis=AX.X)
            part2 = longs.tile([C1, 4], f32, tag="part2")
            for half in range(2):
                mv2 = wk.tile([C1, 2], f32, tag="wk")
                nc.vector.bn_aggr(out=mv2, in_=bnst2[:, half, :, :])
                sqm2 = wk.tile([C1, 1], f32, tag="wk")
                nc.vector.tensor_mul(sqm2, mv2[:, 0:1], mv2[:, 0:1])
                nc.vector.tensor_tensor(out=sqm2, in0=sqm2, in1=mv2[:, 1:2],
                                        op=OP.add)
                nl2 = float(bloc * L2)
                nc.vector.tensor_scalar(out=part2[:, 2 * half:2 * half + 1],
                                        in0=mv2[:, 0:1], scalar1=nl2,
                                        scalar2=None, op0=OP.mult)
                nc.vector.tensor_scalar(out=part2[:, 2 * half + 1:2 * half + 2],
                                        in0=sqm2, scalar1=nl2,
                                        scalar2=None, op0=OP.mult)
            bn2_in = dram.tile([C1, 4], f32, tag="bn2i")
            bn2_out = dram.tile([C1, 4], f32, tag="bn2o")
            nc.sync.dma_start(out=bn2_in, in_=part2)
            nc.gpsimd.collective_compute(
                "AllReduce", OP.add, replica_groups=[list(range(n_cores))],
                ins=[bn2_in[:, :].opt()], outs=[bn2_out[:, :].opt()])
            glob2 = longs.tile([C1, 4], f32, tag="glob2")
            nc.sync.dma_start(out=glob2, in_=bn2_out)

            # feat = relu(sc2 * max_l(conv2) + sh2)   (sc2 > 0 assumed)
            # both halves' scale/shift batched: one Ln+Exp table pair
            g2r = glob2.rearrange("p (h two) -> p h two", two=2)
            mean2 = wk.tile([C1, 2], f32, tag="wk")
            nc.vector.tensor_scalar(out=mean2, in0=g2r[:, :, 0],
                                    scalar1=1.0 / n2, scalar2=None, op0=OP.mult)
            ex22 = wk.tile([C1, 2], f32, tag="wk")
            nc.vector.tensor_scalar(out=ex22, in0=g2r[:, :, 1],
                                    scalar1=1.0 / n2, scalar2=None, op0=OP.mult)
            msq2 = wk.tile([C1, 2], f32, tag="wk")
            nc.vector.tensor_mul(msq2, mean2, mean2)
            nc.vector.tensor_tensor(out=ex22, in0=ex22, in1=msq2,
                                    op=OP.subtract)
            lnv2 = wk.tile([C1, 2], f32, tag="wk")
            nc.scalar.activation(lnv2, ex22, AF.Ln, bias=eps128, scale=1.0)
            rstd2 = wk.tile([C1, 2], f32, tag="wk")
            nc.scalar.activation(rstd2, lnv2, AF.Exp, bias=0.0, scale=-0.5)
            scale2 = longs.tile([C1, 2], f32, tag="bnsc2")
            nc.vector.tensor_mul(scale2, rstd2, bn2g)
            shift2 = longs.tile([C1, 2], f32, tag="bnsh2")
            nc.vector.tensor_mul(shift2, mean2, scale2)
            nc.vector.tensor_tensor(out=shift2, in0=bn2b, in1=shift2,
                                    op=OP.subtract)
            feat = longs.tile([C1, 2 * bloc], f32, tag="feat")
            for half in range(2):
                for b_ in range(bloc):
                    gm = wk.tile([C1, 1], f32, tag="wk")
                    nc.vector.tensor_tensor(out=gm, in0=gmax[:, half, b_, 0:1],
                                            in1=gmax[:, half, b_, 1:2],
                                            op=OP.max)
                    nc.vector.tensor_scalar(
                        out=feat[:, (b_ * 2 + half):(b_ * 2 + half) + 1],
                        in0=gm, scalar1=scale2[:, half:half + 1],
                        scalar2=shift2[:, half:half + 1],
                        op0=OP.mult, op1=OP.add)
                    nc.vector.tensor_scalar(
                        out=feat[:, (b_ * 2 + half):(b_ * 2 + half) + 1],
                        in0=feat[:, (b_ * 2 + half):(b_ * 2 + half) + 1],
                        scalar1=0.0, scalar2=None, op0=OP.max)
            if dbg:
                nc.sync.dma_start(out=dbg_outs["dbg_feat"][:, :], in_=feat)

            # ================= fc =================
            for b_ in range(bloc):
                pf_ = ps_sm.tile([NC, 1], f32, tag="sm")
                for half in range(2):
                    nc.tensor.matmul(pf_, fcwT[:, half * NC:(half + 1) * NC],
                                     feat[:, b_ * 2 + half:b_ * 2 + half + 1],
                                     start=(half == 0), stop=(half == 1))
                ob2 = wk.tile([NC, 1], f32, tag="wk")
                nc.vector.tensor_scalar(out=ob2, in0=pf_, scalar1=fcb_s,
                                        scalar2=None, op0=OP.add)
                nc.sync.dma_start(out=out_d[b_, :], in_=ob2[:, 0])

    # Steer the ACT table-set picker to natural_log_exp_and_others (holds
    # full-size exp AND ln tables) so Ln/Exp alternation never reloads the
    # activation table mid-kernel (each reload costs 1.28us on the exp
    # critical path).
    import concourse.bacc as bacc_mod
    _orig_tables = bacc_mod.get_activation_tables

    def _patched_tables(arch):
        tabs = _orig_tables(arch)
        exp_t = mybir.ActivationFunctionType.Exp
        ln_t = mybir.ActivationFunctionType.Ln
        for name, s in tabs.items():
            if name != "natural_log_exp_and_others":
                s.discard(exp_t)
                s.discard(ln_t)
        return tabs

    bacc_mod.get_activation_tables = _patched_tables
    try:
        nc.compile()
    finally:
        bacc_mod.get_activation_tables = _orig_tables
    return nc


def _host_inputs(inputs, n_cores, bloc, lp):
    X = np.asarray(inputs["X"]).astype(np.float32)[:, :lp]
    sa = np.asarray(inputs["surface_availability"], dtype=np.float32)[:, :lp]
    ptm = np.asarray(inputs["ptm"]).astype(np.float32)[:, :lp]
    emb = np.asarray(inputs["emb"], dtype=np.float32)
    pemb = np.asarray(inputs["ptm_emb"], dtype=np.float32)
    rpe = np.asarray(inputs["rpe"], dtype=np.float32)
    inw = np.asarray(inputs["in_proj_w"], dtype=np.float32)
    inb = np.asarray(inputs["in_proj_b"], dtype=np.float32)
    wo = np.asarray(inputs["out_proj_w"], dtype=np.float32)
    bo = np.asarray(inputs["out_proj_b"], dtype=np.float32)
    w1 = np.asarray(inputs["lin1_w"], dtype=np.float32)
    b1 = np.asarray(inputs["lin1_b"], dtype=np.float32)
    w2 = np.asarray(inputs["lin2_w"], dtype=np.float32)
    b2 = np.asarray(inputs["lin2_b"], dtype=np.float32)
    c1w = np.asarray(inputs["conv1_w"], dtype=np.float32)
    c2w = np.asarray(inputs["conv2_w"], dtype=np.float32)
    fcw = np.asarray(inputs["fc_w"], dtype=np.float32)

    pembp = np.zeros((10, D), np.float32)
    pembp[:, ED:] = pemb
    embp = np.zeros((NCAT, D), np.float32)
    embp[:, :ED] = emb

    clo, chi = rpe[0], rpe[2 * MD]
    bandcat = np.zeros((D, H * BAND_TOT), np.float32)
    jj = np.arange(128)[:, None]
    for h in range(H):
        for di, dl in enumerate(BAND_DELTAS):
            w = BAND_W[di]
            ii = np.arange(BAND_C0[di], BAND_C0[di] + w)[None, :]
            e = dl + jj - ii
            val = rpe[np.clip(e, -MD, MD) + MD, h]
            beta = chi[h] if di == 5 else clo[h]
            bandcat[:, h * BAND_TOT + BAND_OFF[di]:
                    h * BAND_TOT + BAND_OFF[di] + w] = val - beta

    ob_eff = bo + wo @ inb[2 * D:3 * D]
    l2t = w2.T   # [FF, D]

    pf32 = np.zeros((D, PF_W), np.float32)
    pf32[:NCAT, PC_IO25] = np.arange(NCAT, dtype=np.float32)
    pf32[:10, PC_IO10] = np.arange(10, dtype=np.float32)
    pf32[:, PC_QB] = inb[0:D] * ISQ
    pf32[:, PC_KB] = inb[D:2 * D]
    pf32[:, PC_OB] = ob_eff
    pf32[:, PC_L1B:PC_L1B + 2] = b1.reshape(2, D).T
    pf32[:, PC_L2B] = b2
    pf32[:, PC_LN1G] = np.asarray(inputs["ln1_g"], np.float32)
    pf32[:, PC_LN1B] = np.asarray(inputs["ln1_b"], np.float32)
    pf32[:, PC_LN2G] = np.asarray(inputs["ln2_g"], np.float32)
    pf32[:, PC_LN2B] = np.asarray(inputs["ln2_b"], np.float32)
    for h in range(H):
        pf32[:, PC_CEXP + 2 * h] = clo[h]
        pf32[:, PC_CEXP + 2 * h + 1] = chi[h]
    pf32[:, PC_BN1G] = np.asarray(inputs["bn1_g"], np.float32)
    pf32[:, PC_BN1B] = np.asarray(inputs["bn1_b"], np.float32)
    pf32[:, PC_BN2G:PC_BN2G + 2] = np.asarray(inputs["bn2_g"], np.float32).reshape(2, C1).T
    pf32[:, PC_BN2B:PC_BN2B + 2] = np.asarray(inputs["bn2_b"], np.float32).reshape(2, C1).T
    pf32[:NC, PC_FCB] = np.asarray(inputs["fc_b"], np.float32)
    pf32[:, PC_FCW:PC_FCW + 2 * NC] = \
        fcw.T.reshape(2, C1, NC).transpose(1, 0, 2).reshape(C1, 2 * NC)

    import ml_dtypes
    pbig = np.zeros((D, PB_W), ml_dtypes.bfloat16)
    pbig[:NCAT, PB_EMB:PB_EMB + D] = embp
    pbig[:10, PB_PEMB:PB_PEMB + D] = pembp
    pbig[:, PB_INW:PB_INW + 3 * D] = inw.T
    pbig[:, PB_IDENT:PB_IDENT + D] = np.eye(D, dtype=np.float32)
    pbig[:, PB_WO:PB_WO + D] = wo.T
    pbig[:, PB_L1W:PB_L1W + FF] = w1.T
    pbig[:, PB_L2C:PB_L2C + 2 * D] = np.concatenate([l2t[0:D], l2t[D:2 * D]], axis=1)
    pbig[:, PB_C1W:PB_C1W + K * C1] = c1w.transpose(1, 2, 0).reshape(D, K * C1)
    pbig[:, PB_C2W:PB_C2W + K * C2] = c2w.transpose(1, 2, 0).reshape(C1, K * C2)
    pbig[:, PB_BAND:PB_BAND + H * BAND_TOT] = bandcat
    for h in range(H):
        pbig[h, PB_BC4 + HD * h:PB_BC4 + HD * (h + 1)] = 1.0
    for i in range(4):
        pbig[i, PB_SEL + i * D:PB_SEL + (i + 1) * D] = 1.0

    def pack(a):
        return np.ascontiguousarray(a).view(np.uint16).view(np.float32)

    shared = {
        "pf32": pf32,
        "pbig_a": pack(pbig[:, :PB_A]),
        "pbig_b": pack(pbig[:, PB_A:]),
    }
    in_maps = []
    for c in range(n_cores):
        rows = slice(c * bloc, (c + 1) * bloc)
        m = dict(shared)
        m["Xf"] = np.ascontiguousarray(X[rows].reshape(-1))
        m["saf"] = np.ascontiguousarray(sa[rows].reshape(-1))
        m["ptmf"] = np.ascontiguousarray(ptm[rows].reshape(-1))
        in_maps.append(m)
    return in_maps


_NC_CACHE = {}


def _get_nc(n_cores, bloc, lp, dbg=False):
    key = (n_cores, bloc, lp, dbg)
    if key not in _NC_CACHE:
        _NC_CACHE[key] = _build(n_cores, bloc, lp, dbg=dbg)
    return _NC_CACHE[key]


def kernel(**inputs):
    from concourse.bass_utils import run_bass_kernel_spmd
    nc = _get_nc(NCORES, BLOC, L)
    in_maps = _host_inputs(inputs, NCORES, BLOC, L)
    res = run_bass_kernel_spmd(nc, in_maps, list(range(NCORES)))
    out = np.concatenate([res.results[i]["out"] for i in range(NCORES)], axis=0)
    return out.astype(np.float32)
